# revision 3
# baseline (speedup 1.0000x reference)
"""ExpLeak (leaky integrator) Trainium2 kernel.

Computes, over a [B=16, T=1024, N=4096] f32 tensor:
    y[b, t, n] = alpha * y[b, t-1, n] + x[b, t, n],   alpha = exp(-1/tau)

Strategy
--------
Pure data parallel over batch: 8 NeuronCores x 2 batches each.

Per core, the time recurrence is evaluated as a blocked lower-triangular
matmul.  For a time chunk of C=128 steps,

    y_chunk = L @ x_chunk + alphas (x) carry          (outer product)
    L[t, s]    = alpha^(t-s)  for s <= t, else 0
    alphas[t]  = alpha^(t+1)
    carry[n]   = y[last row of previous chunk, n]

Both terms are PE matmuls accumulating into the same PSUM bank:
  - main:  lhsT = L^T  [128,128], rhs = x tile slice [128, 512]
  - carry: lhsT = alphas [1,128], rhs = carry row    [1,   512]  (K=1)
The carry row for the next chunk is the out row 127, moved to partition
0 of an SBUF tile with a small SWDGE DMA.

I/O precision: the kernel is memory-bound (HBM roofline), so x and y
ride HBM as float16 (host casts f32->fp16 round-to-nearest).  The PE
multiplies fp16 at full rate and accumulates in fp32 PSUM; the L
weights are fp16 (e5m10), so the end-to-end rms relative error is
~3e-4 -- inside the 1e-3 target -- while HBM traffic halves vs f32.

I/O layout: the host rearranges x to [NCHUNK, C, B_PER*N] so each time
chunk (both batches fused) is one contiguous 2 MiB block: each load and
store is a single max-efficiency 128-partition DMA (16 KiB contiguous
per partition), and y comes back in the same layout.
"""

import os
import sys

import numpy as np


def _ensure_concourse():
    try:
        import concourse.bass  # noqa: F401
        return
    except ImportError:
        pass
    for p in ("/opt/trn_rl_repo", "/root/.axon_site/_ro/trn_rl_repo"):
        if os.path.isdir(p) and p not in sys.path:
            sys.path.insert(0, p)
    import concourse.bass  # noqa: F401


B, T, N = 16, 1024, 4096
N_CORES = 8
B_PER = B // N_CORES  # batches per core
C = 128               # time chunk (PE contraction dim)
NCHUNK = T // C
FT = 512              # feature tile (max f32 PSUM bank free dim)
NFT = N // FT
W = B_PER * N         # fused free width of one chunk tile

_PROGRAM_CACHE = {}


def build_program(repeats=None, variant="full", io="fp16"):
    """Trace + compile the per-core Bass/Tile program. alpha enters only
    through the lt/av input tensors, so one program serves any tau.

    repeats: if set, wrap the whole body in a tc.For_i loop that redoes
    the identical (idempotent) computation `repeats` times — used by
    test.py to measure the steady-state kernel time as a slope,
    independent of the per-launch dispatch overhead.

    io: "fp16" (default) or "fp32" — dtype of x/y in HBM and of the PE
    operands.  fp32 uses fp32r matmuls with Dekker-split weights."""
    _ensure_concourse()
    import contextlib

    import concourse.bacc as bacc
    import concourse.mybir as mybir
    from concourse import tile

    DT = mybir.dt.float32
    if io == "fp16":
        DIO = mybir.dt.float16   # HBM dtype of x / y
        DPE = mybir.dt.float16   # PE operand dtype
    else:
        DIO = mybir.dt.float32
        DPE = mybir.dt.float32r

    nc = bacc.Bacc("TRN2", target_bir_lowering=False, debug=False,
                   num_devices=N_CORES)
    x = nc.declare_dram_parameter("x", [NCHUNK, C, W], DIO, isOutput=False)
    lt = nc.declare_dram_parameter("lt", [C, C], DIO, isOutput=False)
    ltl = None
    if io == "fp32":
        ltl = nc.declare_dram_parameter("ltl", [C, C], DIO, isOutput=False)
    av = nc.declare_dram_parameter("av", [1, C], DIO, isOutput=False)
    y = nc.declare_dram_parameter("y", [NCHUNK, C, W], DIO, isOutput=True)

    def as_pe(ap):
        return ap.bitcast(DPE) if io == "fp32" else ap

    with tile.TileContext(nc) as tc:
        with (
            tc.tile_pool(name="w", bufs=1) as wpool,
            tc.tile_pool(name="xp", bufs=4) as xpool,
            tc.tile_pool(name="op", bufs=3) as opool,
            tc.tile_pool(name="cp", bufs=4) as cpool,
            tc.tile_pool(name="ps", bufs=8, space="PSUM") as pspool,
        ):
            # fp32 path: the PE reads the top 20 bits (e8m11) of fp32r;
            # weights are pre-rounded on host and L^T is Dekker-split
            # into hi+lo so the main-matmul weights are exact to fp32.
            # fp16 path: weights are plain fp16, single matmul.
            ltt = wpool.tile([C, C], DPE, tag="lt")
            nc.sync.dma_start(ltt[:], as_pe(lt[:]))
            ltlt = None
            if io == "fp32":
                ltlt = wpool.tile([C, C], DPE, tag="ltl")
                nc.sync.dma_start(ltlt[:], as_pe(ltl[:]))
            avt = wpool.tile([1, C], DPE, tag="av")
            nc.sync.dma_start(avt[:], as_pe(av[:]))

            rep = (tc.For_i(0, repeats, 1, staggered_reset=True,
                            hint_engines=(mybir.EngineType.PE,))
                   if repeats else contextlib.nullcontext())
            with rep:
                _emit_body(nc, tc, x, y, xpool, opool, cpool, pspool,
                           ltt, ltlt, avt, DT, DPE, as_pe, mybir, variant)

    nc.compile()
    return nc


def _emit_body(nc, tc, x, y, xpool, opool, cpool, pspool,
               ltt, ltlt, avt, DT, DPE, as_pe, mybir, variant="full"):
    carry = {}
    for k in range(NCHUNK):
        xt = xpool.tile([C, W], DPE, tag="xt")
        # one contiguous 2 MiB load (16 KiB per partition), SP ring
        nc.sync.dma_start(xt[:], as_pe(x[k]))
        if variant == "dma":
            # measurement-only: pure load->store roundtrip
            nc.scalar.dma_start(y[k], as_pe(xt[:]))
            continue
        ot = opool.tile([C, W], DPE, tag="ot")
        for b in range(B_PER):
            newcarry = cpool.tile([1, N], DPE, tag=f"carry{b}")
            for j in range(NFT):
                col = b * N + j * FT
                fsl = slice(col, col + FT)
                ps = pspool.tile([C, FT], DT, tag="ps")
                nc.tensor.matmul(
                    ps[:],
                    ltt[:],
                    xt[:, fsl],
                    start=True,
                    stop=(k == 0 and ltlt is None),
                )
                if ltlt is not None:
                    nc.tensor.matmul(
                        ps[:],
                        ltlt[:],
                        xt[:, fsl],
                        start=False,
                        stop=(k == 0),
                    )
                if k > 0:
                    nc.tensor.matmul(
                        ps[:],
                        avt[:],
                        carry[b][0:1, j * FT:(j + 1) * FT],
                        start=False,
                        stop=True,
                    )
                nc.vector.tensor_copy(ot[:, fsl], ps[:])
            # next chunk's carry: out row 127 -> partition 0 (SWDGE
            # keeps this dependent little DMA out of the HWDGE FIFOs;
            # per-batch so it fires as soon as that batch's copies land)
            nc.gpsimd.dma_start(newcarry[0:1, :],
                                ot[C - 1:C, b * N:(b + 1) * N])
            carry[b] = newcarry
        # one contiguous 2 MiB store on the ACT HWDGE ring, so the SP
        # ring only carries loads and streams ahead (measured best)
        nc.scalar.dma_start(y[k], as_pe(ot[:]))


def _get_program():
    nc = _PROGRAM_CACHE.get("nc")
    if nc is None:
        nc = build_program()
        _PROGRAM_CACHE["nc"] = nc
    return nc


def _round_fp32r(a: np.ndarray) -> np.ndarray:
    """Round fp32 to the PE's fp32r grid (e8m11: low 12 mantissa bits
    zero), round-to-nearest-even."""
    bits = a.astype(np.float32).view(np.uint32)
    keep = np.uint32(0xFFFFF000)
    low = bits & np.uint32(0xFFF)
    lsb = (bits >> np.uint32(12)) & np.uint32(1)
    round_up = (low > 0x800) | ((low == 0x800) & (lsb == 1))
    out = (bits & keep) + np.where(round_up, np.uint32(0x1000), np.uint32(0))
    return out.view(np.float32)


def make_weights(alpha: float, io="fp16"):
    """Host-side constant tensors.
    fp16: lt = L^T and av[0,t] = alpha^(t+1), both rounded to fp16.
    fp32: lt/ltl = hi/lo Dekker split of L^T on the fp32r grid."""
    powers = np.power(np.float64(alpha), np.arange(C + 1))
    lt = np.zeros((C, C), dtype=np.float32)
    s_idx, t_idx = np.meshgrid(np.arange(C), np.arange(C), indexing="ij")
    mask = s_idx <= t_idx
    lt[mask] = powers[(t_idx - s_idx)[mask]].astype(np.float32)
    av = powers[1:].astype(np.float32).reshape(1, C)
    if io == "fp16":
        return lt.astype(np.float16), None, av.astype(np.float16)
    lt_hi = _round_fp32r(lt)
    lt_lo = _round_fp32r((lt - lt_hi).astype(np.float32))
    return lt_hi, lt_lo, _round_fp32r(av)


def _to_chunked(xc: np.ndarray) -> np.ndarray:
    """[B_PER, T, N] -> [NCHUNK, C, B_PER*N] (chunk-contiguous layout)."""
    return np.ascontiguousarray(
        xc.reshape(B_PER, NCHUNK, C, N).transpose(1, 2, 0, 3)
    ).reshape(NCHUNK, C, W)


def _from_chunked(yc: np.ndarray) -> np.ndarray:
    """[NCHUNK, C, B_PER*N] -> [B_PER, T, N]."""
    return np.ascontiguousarray(
        yc.reshape(NCHUNK, C, B_PER, N).transpose(2, 0, 1, 3)
    ).reshape(B_PER, T, N)


def prepare_in_maps(input_current: np.ndarray, tau_mem: np.ndarray,
                    io="fp16"):
    """Shard + cast + rearrange the full inputs into per-core dicts."""
    tau = np.float32(np.asarray(tau_mem).reshape(-1)[0])
    alpha = float(np.exp(np.float32(-1.0) / tau))
    lt, ltl, av = make_weights(alpha, io=io)
    x = np.ascontiguousarray(input_current, dtype=np.float32)
    if io == "fp16":
        x = x.astype(np.float16)
    else:
        # round-to-nearest onto the fp32r grid (instead of the PE's
        # truncation of the low 12 bits: halves the input error)
        x = _round_fp32r(x)
    maps = []
    for c in range(N_CORES):
        m = {"x": _to_chunked(x[c * B_PER:(c + 1) * B_PER]),
             "lt": lt, "av": av}
        if ltl is not None:
            m["ltl"] = ltl
        maps.append(m)
    return maps


def kernel(input_current: np.ndarray, tau_mem: np.ndarray) -> np.ndarray:
    _ensure_concourse()
    from concourse.bass_utils import run_bass_kernel_spmd

    nc = _get_program()
    in_maps = prepare_in_maps(input_current, tau_mem, io="fp16")
    res = run_bass_kernel_spmd(nc, in_maps, list(range(N_CORES)))
    out = np.concatenate(
        [_from_chunked(res.results[c]["y"]) for c in range(N_CORES)], axis=0)
    return out.astype(np.float32, copy=False)


# revision 7
# speedup vs baseline: 1.8594x; 1.8594x over previous
"""ExpLeak (leaky integrator) Trainium2 kernel.

Computes, over a [B=16, T=1024, N=4096] f32 tensor:
    y[b, t, n] = alpha * y[b, t-1, n] + x[b, t, n],   alpha = exp(-1/tau)

Strategy
--------
Pure data parallel: 8 NeuronCores x 2 batches each; within a core the
(batch, feature) rows are independent recurrences over time.

The host transposes x to row-major [rows = B_PER*N, T] so that TIME is
the free (column) axis, then the whole module is a single ISA op per
tile: ``tensor_tensor_scan`` (TensorTensorScanArith, 0xe5) runs
``state = alpha * state + x[:, t]`` along the free dimension with one
independent fp32 state per partition.  No matmuls, no cross-chunk carry
chain, no PE at all -- each [128, T] tile is load -> scan -> store with
no other dependencies, so the kernel streams at the HBM roofline.

I/O precision: x and y ride HBM as float16 (host casts round-to-
nearest); the scan state is fp32 internally and alpha is given as an
fp32 operand, so the end-to-end rms relative error is ~3e-4 (fp16
input + output quantization only) -- inside the 1e-3 target -- while
HBM traffic halves vs f32 (16 MiB -> 8 MiB per core each way... i.e.
32 MiB total per core with both directions).

Layout: per core x is [GB=8 blocks, 128 partitions, GW=8192] fp16 --
each block one contiguous 2 MiB DMA (16 KiB per partition).  Columns
[g*1024, (g+1)*1024) of partition p hold row (1024*blk + 8*p + g)'s
full time series, so each block is 8 independent [128, 1024] scans,
alternated across the DVE and GpSimd engines.
"""

import os
import sys

import numpy as np


def _ensure_concourse():
    try:
        import concourse.bass  # noqa: F401
        return
    except ImportError:
        pass
    for p in ("/opt/trn_rl_repo", "/root/.axon_site/_ro/trn_rl_repo"):
        if os.path.isdir(p) and p not in sys.path:
            sys.path.insert(0, p)
    import concourse.bass  # noqa: F401


B, T, N = 16, 1024, 4096
N_CORES = 8
B_PER = B // N_CORES      # batches per core
ROWS = B_PER * N          # independent scan rows per core
GPB = 8                   # row-groups per block (scans per block)
GW = GPB * T              # free width of one block tile (8192)
GB = ROWS // (128 * GPB)  # blocks per core (8)

_PROGRAM_CACHE = {}


def build_program(repeats=None, variant="full", io="fp16"):
    """Trace + compile the per-core Bass/Tile program.  alpha enters
    only through the ac input tensor, so one program serves any tau.

    repeats: if set, wrap the whole body in a tc.For_i loop that redoes
    the identical (idempotent) computation `repeats` times — used by
    test.py to measure the steady-state kernel time as a slope,
    independent of the per-launch dispatch overhead."""
    _ensure_concourse()
    import contextlib

    import concourse.bacc as bacc
    import concourse.mybir as mybir
    from concourse import tile

    assert io == "fp16"
    DIO = mybir.dt.float16

    nc = bacc.Bacc("TRN2", target_bir_lowering=False, debug=False,
                   num_devices=N_CORES)
    x = nc.declare_dram_parameter("x", [GB, 128, GW], DIO, isOutput=False)
    # alpha as an fp32 operand tile: fp16 alpha would perturb the decay
    # rate by up to 2.4e-4, which the T-step recurrence amplifies ~14x.
    # Zeros at row boundaries (w % T == 0) reset the recurrence exactly
    # (state = 0*prev + x_0), letting ONE scan instruction cover a whole
    # block (8 chained rows per partition).
    ac = nc.declare_dram_parameter("ac", [128, GW], mybir.dt.float32,
                                   isOutput=False)
    y = nc.declare_dram_parameter("y", [GB, 128, GW], DIO, isOutput=True)

    with tile.TileContext(nc) as tc:
        with (
            tc.tile_pool(name="w", bufs=1) as wpool,
            tc.tile_pool(name="xp", bufs=3) as xpool,
            tc.tile_pool(name="op", bufs=3) as opool,
        ):
            act = wpool.tile([128, GW], mybir.dt.float32, tag="ac")
            nc.sync.dma_start(act[:], ac[:])

            rep = (tc.For_i(0, repeats, 1, staggered_reset=True,
                            hint_engines=(mybir.EngineType.DVE,))
                   if repeats else contextlib.nullcontext())
            with rep:
                _emit_body(nc, tc, x, y, xpool, opool, act, DIO, mybir,
                           variant)

    nc.compile()
    return nc


def _emit_body(nc, tc, x, y, xpool, opool, act, DIO, mybir, variant="full"):
    for g in range(GB):
        xt = xpool.tile([128, GW], DIO, tag="xt")
        # one contiguous 2 MiB load (16 KiB per partition), SP ring
        nc.sync.dma_start(xt[:], x[g])
        if variant == "dma":
            # measurement-only: pure load->store roundtrip
            nc.scalar.dma_start(y[g], xt[:])
            continue
        ot = opool.tile([128, GW], DIO, tag="ot")
        # one DVE scan per block: state = ac[:,w] * state + x[:,w]; the
        # zeros in ac at row boundaries reset the recurrence, so the 8
        # rows chained per partition come out exact (scan is DVE-only —
        # the Pool engine rejects TensorTensorScanArith)
        nc.vector.tensor_tensor_scan(
            ot[:],
            act[:],
            xt[:],
            0.0,
            mybir.AluOpType.mult,
            mybir.AluOpType.add,
        )
        # one contiguous 2 MiB store on the ACT HWDGE ring, so the SP
        # ring only carries loads and streams ahead
        nc.scalar.dma_start(y[g], ot[:])


def _get_program():
    nc = _PROGRAM_CACHE.get("nc")
    if nc is None:
        nc = build_program()
        _PROGRAM_CACHE["nc"] = nc
    return nc


def _to_rows(xc: np.ndarray) -> np.ndarray:
    """[B_PER, T, N] (any float dtype) -> [GB, 128, GW] fp16 rows-major
    time series: row r = (b, n) = (r // N, r % N), laid out so block g,
    partition p, columns [s*T, (s+1)*T) hold row 1024*g + 8*p + s."""
    rows = np.ascontiguousarray(
        xc.transpose(0, 2, 1), dtype=np.float16)        # [B_PER, N, T]
    return rows.reshape(GB, 128, GW)


def _from_rows(yr: np.ndarray) -> np.ndarray:
    """[GB, 128, GW] -> [B_PER, T, N] float32."""
    rows = yr.reshape(B_PER, N, T)
    return np.ascontiguousarray(rows.transpose(0, 2, 1), dtype=np.float32)


def prepare_in_maps(input_current: np.ndarray, tau_mem: np.ndarray,
                    io="fp16"):
    """Shard + cast + transpose the full inputs into per-core dicts."""
    tau = np.float32(np.asarray(tau_mem).reshape(-1)[0])
    alpha = np.float32(np.exp(np.float32(-1.0) / tau))
    ac = np.full((128, GW), alpha, dtype=np.float32)
    ac[:, ::T] = 0.0  # reset the chained recurrence at each row start
    x = np.asarray(input_current)
    maps = []
    for c in range(N_CORES):
        maps.append({"x": _to_rows(x[c * B_PER:(c + 1) * B_PER]), "ac": ac})
    return maps


def kernel(input_current: np.ndarray, tau_mem: np.ndarray) -> np.ndarray:
    _ensure_concourse()
    from concourse.bass_utils import run_bass_kernel_spmd

    nc = _get_program()
    in_maps = prepare_in_maps(input_current, tau_mem)
    res = run_bass_kernel_spmd(nc, in_maps, list(range(N_CORES)))
    out = np.concatenate(
        [_from_rows(res.results[c]["y"]) for c in range(N_CORES)], axis=0)
    return out
